# revision 3
# baseline (speedup 1.0000x reference)
"""Trainium2 Bass kernel for the Deep Lagrangian Network problem.

Strategy (pure data-parallel over 8 cores, 4096 samples/core):
  Phase 1 (feature-major, samples on the free dim):
    - MLP trunk + heads as fp32r matmuls.  softplus is computed as
      -Ln(1 - sigmoid(z)) with the minus sign folded into host-negated
      weights of the next layer, so the ACT engine only needs the
      {sigmoid, tanh} and {ln, exp} table sets (~2 loads per layer).
    - Forward-mode Jacobian chain through W1/W2 with 7 tangents as fp32r
      matmuls [256,256] @ [256, Ns] per tangent, sigmoid-scaled on DVE
      (one merged [128,1024] op per layer per tangent).
    - The tril part of the head-jacobian weights is pre-expanded to the full
      49 L-rows on the host, so dL/dq assembles directly in PSUM; the diag
      (softplus') scaling is deferred to cheap sample-major fix-ups.
    - 0/1 "scatter" matmuls assemble [L | g | Ld_raw | sld] feature-major;
      PE transposes flip everything to sample-major.
  Phase 2 (sample-major, 128 partitions = samples, 4 groups along free):
    - The tiny per-sample einsums as broadcast/strided DVE/GPSIMD tensor ops
      using the factored Coriolis form:
        u = L^T qd, V[m,c] = sum_j qd_j dL[j,m]/dq_c, w = V qd,
        Ldt = dL/dq . qd, c = L w + Ldt u - u^T V, Hm = L L^T,
        tau = Hm qdd + c + g + tau_fric.
"""

import sys

if "/opt/trn_rl_repo" not in sys.path:
    sys.path.insert(0, "/opt/trn_rl_repo")

from contextlib import ExitStack

import numpy as np

import concourse.bacc as bacc
import concourse.bass as bass
import concourse.tile as tile
from concourse import mybir

f32 = mybir.dt.float32
f32r = mybir.dt.float32r
AF = mybir.ActivationFunctionType
ALU = mybir.AluOpType
AX = mybir.AxisListType

N = 7
NTRIL = 21
HID = 256
B_FULL = 32768
N_CORES = 8
BC = B_FULL // N_CORES  # samples per core
NS = 512                # samples per supertile
NT = BC // NS           # supertiles per core

# sample-major field map (per group, stride SMW)
#   0:49    L (full 7x7, row-major i*7+j)
#   49:56   g
#   56:63   Ld_raw
#   63:70   sld = sigmoid(Ld_raw)
#   70 + 56*c + r   D-block for tangent c:
#       r in [0,49): dLtril[i,j]/dq_c at r = 7i+j (zeros at diag/upper)
#       r in [49,56): unscaled diag jac row d (= Wld_d . J2)
DBW = 56
SMW = 70 + DBW * N  # 462


def _build_program(bc=BC, ns=NS, repeat=None):
    nt = bc // ns
    g_per = ns // 128
    nsh = ns // 2
    nc = bacc.Bacc("TRN2", target_bir_lowering=False, debug=False)

    dr = {}

    def din(name, shape, dt=f32):
        dr[name] = nc.dram_tensor(name, list(shape), dt, kind="ExternalInput")
        return dr[name]

    def dout(name, shape, dt=f32):
        dr[name] = nc.dram_tensor(name, list(shape), dt, kind="ExternalOutput")
        return dr[name]

    din("q", [bc, N]); din("qd", [bc, N]); din("qdd", [bc, N])
    din("w0t", [N, HID], f32r)          # W0.T                (true sign)
    din("w0c", [HID, N])                # W0 columns          (true sign)
    din("w1tn", [HID, HID], f32r)       # -W1.T  (forward, consumes a'=-A)
    din("w2tn", [HID, HID], f32r)       # -W2.T
    din("w1tj", [HID, HID], f32r)       # W1.T   (jacobian chain)
    din("w2tj", [HID, HID], f32r)       # W2.T
    din("wldtn", [HID, N], f32r)        # -Wld.T (forward diag head)
    din("wlxt", [HID, 70], f32r)        # folded fwd head -> Lx rows 0:63
    din("whjt", [HID, DBW], f32r)       # head-jac: [Wlt expanded to 49 | Wld]
    din("b0", [HID]); din("b1", [HID]); din("b2", [HID])
    din("bld", [N])
    din("s_ld", [N, 70], f32r)          # diag scatter, entries -1
    din("s_sld", [N, 70], f32r)         # sld rows 63:70
    din("s_bias", [1, 70], f32r)        # bias row (blt/bg/bld) + 1e-3 diag
    din("ident", [128, 128])
    din("ones_row", [1, ns], f32r)
    din("fricC", [128, 4 * N])          # [fd | fc | fs | -1/fv_c] replicated

    dout("o_tau", [bc, N]); dout("o_hm", [bc, 49]); dout("o_c", [bc, N])
    dout("o_g", [bc, N]); dout("o_fric", [bc, N]); dout("o_ldraw", [bc, N])

    with tile.TileContext(nc) as tc, ExitStack() as ctx:
        sb = ctx.enter_context(tc.tile_pool(name="sb", bufs=2))
        cst = ctx.enter_context(tc.tile_pool(name="cst", bufs=1))
        ps = ctx.enter_context(tc.tile_pool(name="ps", bufs=1, space="PSUM"))

        # ---- constants / weights (loaded once) ----
        w0t_sb = cst.tile([N, HID], f32r)
        nc.sync.dma_start(w0t_sb[:], dr["w0t"].ap()[:, :])
        w0c_sb = [cst.tile([128, N], f32, tag=f"w0c{m}", name=f"w0c{m}") for m in range(2)]
        for m in range(2):
            nc.sync.dma_start(w0c_sb[m][:], dr["w0c"].ap()[128 * m:128 * (m + 1), :])
        wsb = {}
        for nm in ("w1tn", "w2tn", "w1tj", "w2tj"):
            for k in range(2):
                for m in range(2):
                    t = cst.tile([128, 128], f32r, tag=f"{nm}{k}{m}", name=f"{nm}{k}{m}")
                    nc.sync.dma_start(t[:], dr[nm].ap()[128 * k:128 * (k + 1), 128 * m:128 * (m + 1)])
                    wsb[nm, k, m] = t
        for nm, w in (("wldtn", N), ("wlxt", 70), ("whjt", DBW)):
            for k in range(2):
                t = cst.tile([128, w], f32r, tag=f"{nm}{k}", name=f"{nm}{k}")
                nc.sync.dma_start(t[:], dr[nm].ap()[128 * k:128 * (k + 1), :])
                wsb[nm, k] = t
        b_sb = {}
        for nm in ("b0", "b1", "b2"):
            for m in range(2):
                t = cst.tile([128, 1], f32, tag=f"{nm}_{m}", name=f"{nm}_{m}")
                nc.sync.dma_start(t[:], dr[nm].ap()[128 * m:128 * (m + 1)].unsqueeze(1))
                b_sb[nm, m] = t
        bld_sb = cst.tile([N, 1], f32)
        nc.sync.dma_start(bld_sb[:], dr["bld"].ap()[:].unsqueeze(1))
        scat = {}
        for nm, p in (("s_ld", N), ("s_sld", N), ("s_bias", 1)):
            t = cst.tile([p, 70], f32r, tag=nm, name=nm)
            nc.sync.dma_start(t[:], dr[nm].ap()[:, :])
            scat[nm] = t
        ident = cst.tile([128, 128], f32)
        nc.sync.dma_start(ident[:], dr["ident"].ap()[:, :])
        ones_sb = cst.tile([1, ns], f32r)
        nc.sync.dma_start(ones_sb[:], dr["ones_row"].ap()[:, :])
        fricC = cst.tile([128, 4 * N], f32)
        nc.sync.dma_start(fricC[:], dr["fricC"].ap()[:, :])

        def body(t):
            r0 = ns * t

            # ---------------- input loads (sample s = r0 + 4p + g) ----------
            qsm = sb.tile([128, g_per * N], f32, tag="qsm", name="qsm")
            nc.sync.dma_start(qsm[:], dr["q"].ap()[r0:r0 + ns, :].rearrange("(p g) n -> p (g n)", g=g_per))
            qdsm = sb.tile([128, g_per * N], f32, tag="qdsm", name="qdsm")
            nc.sync.dma_start(qdsm[:], dr["qd"].ap()[r0:r0 + ns, :].rearrange("(p g) n -> p (g n)", g=g_per))
            qddsm = sb.tile([128, g_per * N], f32, tag="qddsm", name="qddsm")
            nc.sync.dma_start(qddsm[:], dr["qdd"].ap()[r0:r0 + ns, :].rearrange("(p g) n -> p (g n)", g=g_per))

            w7 = g_per * N
            qd3 = qdsm[:].rearrange("p (g n) -> p g n", g=g_per)
            qdd3 = qddsm[:].rearrange("p (g n) -> p g n", g=g_per)
            fr = {nm: fricC[:, N * ix:N * (ix + 1)].unsqueeze(1).broadcast_to([128, g_per, N])
                  for ix, nm in enumerate(("fd", "fc", "fs", "nifv"))}

            def vtile(w, tg):
                return sb.tile([128, g_per * w], f32, tag=tg, name=tg)

            # friction pieces for the ln/exp table window
            qd2 = vtile(N, "qd2")
            nc.gpsimd.tensor_tensor(out=qd2[:], in0=qdsm[:, 0:w7], in1=qdsm[:, 0:w7], op=ALU.mult)
            ei = vtile(N, "ei")
            nc.gpsimd.tensor_tensor(out=ei[:].rearrange("p (g n) -> p g n", g=g_per),
                                    in0=qd2[:].rearrange("p (g n) -> p g n", g=g_per),
                                    in1=fr["nifv"], op=ALU.mult)
            ee = vtile(N, "ee")
            nc.scalar.activation(ee[:], ei[:], AF.Exp)

            # q to feature-major [7, ns] via PE transposes
            qt_ps = ps.tile([N, ns], f32, tag="hd", bufs=2, name="qt_ps")
            for g in range(g_per):
                nc.tensor.transpose(qt_ps[:, 128 * g:128 * (g + 1)], qsm[:, N * g:N * (g + 1)], ident[:, :])
            qt = sb.tile([N, ns], f32, tag="qt", name="qt")
            nc.scalar.activation(qt[:].bitcast(f32r), qt_ps[:], AF.Copy)

            # ---------------- MLP trunk ------------------------------------
            # a' = Ln(1 - sigmoid(z+b)) = -softplus(z+b); sign folded into
            # the (negated) next-layer weights.  m-tiles merged along free.
            aneg = {}
            sig = {}
            for li in range(3):
                z_ps = ps.tile([128, 2 * ns], f32, tag="zjp", bufs=3, name=f"z{li}")
                for m in range(2):
                    zv = z_ps[:, ns * m:ns * (m + 1)]
                    if li == 0:
                        nc.tensor.matmul(zv, w0t_sb[:, 128 * m:128 * (m + 1)],
                                         qt[:].bitcast(f32r), start=True, stop=True)
                    else:
                        for k in range(2):
                            nc.tensor.matmul(zv, wsb[f"w{li}tn", k, m][:],
                                             aneg[li - 1][:, ns * k:ns * (k + 1)].bitcast(f32r),
                                             start=(k == 0), stop=(k == 1))
                s = sb.tile([128, 2 * ns], f32, tag=f"s{li}", name=f"s{li}")
                for m in range(2):
                    nc.scalar.activation(s[:, ns * m:ns * (m + 1)], z_ps[:, ns * m:ns * (m + 1)],
                                         AF.Sigmoid, bias=b_sb[f"b{li}", m][:])
                a = sb.tile([128, 2 * ns], f32, tag=f"a{li}", name=f"a{li}")
                nc.scalar.activation(a[:].bitcast(f32r), s[:], AF.Ln, bias=1.0, scale=-1.0)
                sig[li] = s
                aneg[li] = a

            # ---------------- heads (forward) ------------------------------
            ld_ps = ps.tile([N, ns], f32, tag="hd", bufs=2, name="ld_ps")
            for k in range(2):
                nc.tensor.matmul(ld_ps[:], wsb["wldtn", k][:],
                                 aneg[2][:, ns * k:ns * (k + 1)].bitcast(f32r),
                                 start=(k == 0), stop=(k == 1))
            sld = sb.tile([N, ns], f32, tag="sld", name="sld")
            nc.scalar.activation(sld[:].bitcast(f32r), ld_ps[:], AF.Sigmoid, bias=bld_sb[:])
            ldspn = sb.tile([N, ns], f32, tag="ldspn", name="ldspn")  # = -softplus(Ld_raw)
            nc.scalar.activation(ldspn[:].bitcast(f32r), sld[:], AF.Ln, bias=1.0, scale=-1.0)

            # Lx = [L(49) | g(7) | Ld_raw(7) | sld(7)]
            lx_ps = ps.tile([70, ns], f32, tag="hd", bufs=2, name="lx_ps")
            for k in range(2):
                nc.tensor.matmul(lx_ps[:], wsb["wlxt", k][:],
                                 aneg[2][:, ns * k:ns * (k + 1)].bitcast(f32r),
                                 start=(k == 0), stop=False)
            nc.tensor.matmul(lx_ps[:], scat["s_ld"][:], ldspn[:].bitcast(f32r), start=False, stop=False)
            nc.tensor.matmul(lx_ps[:], scat["s_sld"][:], sld[:].bitcast(f32r), start=False, stop=False)
            nc.tensor.matmul(lx_ps[:], scat["s_bias"][:], ones_sb[:], start=False, stop=True)
            lxsb = sb.tile([70, ns], f32, tag="lxsb", name="lxsb")
            nc.scalar.copy(lxsb[:], lx_ps[:])

            # ---------------- Jacobian chain (per tangent c) ----------------
            dfm = sb.tile([DBW, N * ns], f32, tag="dfm", bufs=1, name="dfm")
            for c in range(N):
                j0 = sb.tile([128, 2 * ns], f32, tag="j0", name="j0")
                for m in range(2):
                    nc.gpsimd.tensor_scalar(out=j0[:, ns * m:ns * (m + 1)].bitcast(f32r),
                                            in0=sig[0][:, ns * m:ns * (m + 1)],
                                            scalar1=w0c_sb[m][:, c:c + 1], scalar2=None, op0=ALU.mult)
                jcur = j0
                jnext = {}
                for li in (1, 2):
                    jp = ps.tile([128, 2 * ns], f32, tag="zjp", bufs=3, name=f"jp{li}")
                    for m in range(2):
                        for k in range(2):
                            nc.tensor.matmul(jp[:, ns * m:ns * (m + 1)],
                                             wsb[f"w{li}tj", k, m][:],
                                             jcur[:, ns * k:ns * (k + 1)].bitcast(f32r),
                                             start=(k == 0), stop=(k == 1))
                    jn = sb.tile([128, 2 * ns], f32, tag=f"j{li}", name=f"j{li}")
                    nc.vector.tensor_tensor(out=jn[:].bitcast(f32r), in0=jp[:], in1=sig[li][:], op=ALU.mult)
                    jcur = jn
                d_ps = ps.tile([DBW, ns], f32, tag="hd", bufs=2, name="d_ps")
                for k in range(2):
                    nc.tensor.matmul(d_ps[:], wsb["whjt", k][:],
                                     jcur[:, ns * k:ns * (k + 1)].bitcast(f32r),
                                     start=(k == 0), stop=(k == 1))
                nc.scalar.copy(dfm[:, ns * c:ns * (c + 1)], d_ps[:])

            # ---------------- transpose to sample-major ---------------------
            smt = sb.tile([128, g_per * SMW], f32, tag="smt", name="smt")
            for g in range(g_per):
                st_ps = ps.tile([128, SMW], f32, tag="hd", bufs=2, name="st_ps")
                nc.tensor.transpose(st_ps[:, 0:70], lxsb[:, 128 * g:128 * (g + 1)], ident[0:70, 0:70])
                for c in range(N):
                    nc.tensor.transpose(st_ps[:, 70 + DBW * c:70 + DBW * (c + 1)],
                                        dfm[:, ns * c + 128 * g:ns * c + 128 * (g + 1)],
                                        ident[0:DBW, 0:DBW])
                if g % 2 == 0:
                    nc.vector.tensor_copy(smt[:, SMW * g:SMW * (g + 1)], st_ps[:])
                else:
                    nc.scalar.copy(smt[:, SMW * g:SMW * (g + 1)], st_ps[:])

            # ---------------- phase 2: per-sample einsums -------------------
            P = 128
            sm3 = smt[:].rearrange("p (g f) -> p g f", g=g_per)
            qdB_mj = qd3.unsqueeze(2).broadcast_to([P, g_per, N, N])  # value qd[innermost]

            # u[m] = sum_j qd[j] L[j,m]
            p1 = vtile(49, "p1")
            nc.gpsimd.tensor_tensor(out=p1[:].rearrange("p (g m j) -> p g m j", g=g_per, m=N),
                                    in0=sm3[:, :, 0:49].rearrange("p g (j m) -> p g m j", j=N),
                                    in1=qdB_mj, op=ALU.mult)
            u_t = vtile(N, "u_t")
            nc.vector.tensor_reduce(out=u_t[:].rearrange("p (g m) -> p g m", g=g_per),
                                    in_=p1[:].rearrange("p (g m j) -> p g m j", g=g_per, m=N),
                                    axis=AX.X, op=ALU.add)
            # V[m,c] = sum_j qd[j] Dtril[c][j,m]   (per-group: 3 free dims)
            p2 = vtile(343, "p2")
            p2v = p2[:].rearrange("p (g x) -> p g x", g=g_per)
            for g in range(g_per):
                Dg = sm3[:, g, 70:SMW].rearrange("p (c r) -> p c r", c=N)[:, :, 0:49]
                nc.gpsimd.tensor_tensor(out=p2v[:, g, :].rearrange("p (m c j) -> p m c j", m=N, c=N),
                                        in0=Dg.rearrange("p c (j m) -> p m c j", j=N),
                                        in1=qd3[:, g, :].unsqueeze(1).unsqueeze(2).broadcast_to([P, N, N, N]),
                                        op=ALU.mult)
            v_t = vtile(49, "v_t")
            nc.vector.tensor_reduce(out=v_t[:].rearrange("p (g m c) -> p g m c", g=g_per, m=N),
                                    in_=p2[:].rearrange("p (g m c j) -> p g m c j", g=g_per, m=N, c=N),
                                    axis=AX.X, op=ALU.add)
            # w[m] = sum_c V[m,c] qd[c]  (tril part)
            p3 = vtile(49, "p3")
            nc.gpsimd.tensor_tensor(out=p3[:].rearrange("p (g m c) -> p g m c", g=g_per, m=N),
                                    in0=v_t[:].rearrange("p (g m c) -> p g m c", g=g_per, m=N),
                                    in1=qdB_mj, op=ALU.mult)
            w_t = vtile(N, "w_t")
            nc.vector.tensor_reduce(out=w_t[:].rearrange("p (g m) -> p g m", g=g_per),
                                    in_=p3[:].rearrange("p (g m c) -> p g m c", g=g_per, m=N),
                                    axis=AX.X, op=ALU.add)
            # Ldt[i,m] = sum_c Dtril[c][i,m] qd[c]  (per-group)
            p4 = vtile(343, "p4")
            p4v = p4[:].rearrange("p (g x) -> p g x", g=g_per)
            for g in range(g_per):
                Dg = sm3[:, g, 70:SMW].rearrange("p (c r) -> p c r", c=N)[:, :, 0:49]
                nc.gpsimd.tensor_tensor(out=p4v[:, g, :].rearrange("p (i m c) -> p i m c", i=N, m=N),
                                        in0=Dg.rearrange("p c (i m) -> p i m c", i=N),
                                        in1=qd3[:, g, :].unsqueeze(1).unsqueeze(2).broadcast_to([P, N, N, N]),
                                        op=ALU.mult)
            ldt_t = vtile(49, "ldt_t")
            nc.vector.tensor_reduce(out=ldt_t[:].rearrange("p (g i m) -> p g i m", g=g_per, i=N),
                                    in_=p4[:].rearrange("p (g i m c) -> p g i m c", g=g_per, i=N, m=N),
                                    axis=AX.X, op=ALU.add)
            # diag fix-up ingredients:
            #   jdraw[c][m] (unscaled diag jac), sld, qs = qd*sld,
            #   jq[m] = sum_c jdraw[c][m] qd[c], jqs = sld*jq
            sldv = sm3[:, :, 63:70]
            qs = vtile(N, "qs")
            nc.vector.tensor_tensor(out=qs[:].rearrange("p (g n) -> p g n", g=g_per),
                                    in0=qd3, in1=sldv, op=ALU.mult)
            pjq = vtile(49, "pjq")
            nc.gpsimd.tensor_tensor(out=pjq[:].rearrange("p (g m c) -> p g m c", g=g_per, m=N),
                                    in0=sm3[:, :, 70:SMW].rearrange("p g (c r) -> p g r c", c=N)[:, :, 49:56, :],
                                    in1=qdB_mj, op=ALU.mult)
            jq = vtile(N, "jq")
            nc.vector.tensor_reduce(out=jq[:].rearrange("p (g m) -> p g m", g=g_per),
                                    in_=pjq[:].rearrange("p (g m c) -> p g m c", g=g_per, m=N),
                                    axis=AX.X, op=ALU.add)
            jqs = vtile(N, "jqs")
            nc.vector.tensor_tensor(out=jqs[:].rearrange("p (g n) -> p g n", g=g_per),
                                    in0=jq[:].rearrange("p (g n) -> p g n", g=g_per),
                                    in1=sldv, op=ALU.mult)
            # w full: w2 = w_t + qd*jqs
            wf = vtile(N, "wf")
            nc.gpsimd.tensor_tensor(out=wf[:], in0=qdsm[:, 0:w7], in1=jqs[:], op=ALU.mult)
            w2 = vtile(N, "w2")
            nc.vector.tensor_tensor(out=w2[:], in0=w_t[:], in1=wf[:], op=ALU.add)
            # Ldtu[i] = sum_m Ldt[i,m] u[m]  + jqs[i]*u[i]
            p5 = vtile(49, "p5")
            nc.gpsimd.tensor_tensor(out=p5[:].rearrange("p (g i m) -> p g i m", g=g_per, i=N),
                                    in0=ldt_t[:].rearrange("p (g i m) -> p g i m", g=g_per, i=N),
                                    in1=u_t[:].rearrange("p (g m) -> p g m", g=g_per).unsqueeze(2).broadcast_to([P, g_per, N, N]),
                                    op=ALU.mult)
            ldtu = vtile(N, "ldtu")
            nc.vector.tensor_reduce(out=ldtu[:].rearrange("p (g i) -> p g i", g=g_per),
                                    in_=p5[:].rearrange("p (g i m) -> p g i m", g=g_per, i=N),
                                    axis=AX.X, op=ALU.add)
            lff = vtile(N, "lff")
            nc.gpsimd.tensor_tensor(out=lff[:], in0=jqs[:], in1=u_t[:], op=ALU.mult)
            ldtu2 = vtile(N, "ldtu2")
            nc.vector.tensor_tensor(out=ldtu2[:], in0=ldtu[:], in1=lff[:], op=ALU.add)
            # Lw[i] = sum_m L[i,m] w2[m]
            p6 = vtile(49, "p6")
            nc.gpsimd.tensor_tensor(out=p6[:].rearrange("p (g i m) -> p g i m", g=g_per, i=N),
                                    in0=sm3[:, :, 0:49].rearrange("p g (i m) -> p g i m", i=N),
                                    in1=w2[:].rearrange("p (g m) -> p g m", g=g_per).unsqueeze(2).broadcast_to([P, g_per, N, N]),
                                    op=ALU.mult)
            lw = vtile(N, "lw")
            nc.vector.tensor_reduce(out=lw[:].rearrange("p (g i) -> p g i", g=g_per),
                                    in_=p6[:].rearrange("p (g i m) -> p g i m", g=g_per, i=N),
                                    axis=AX.X, op=ALU.add)
            # uV[c] = sum_m u[m] V[m,c]  (tril) + sum_m u[m] qs[m] jdraw[c][m]
            p7 = vtile(49, "p7")
            nc.gpsimd.tensor_tensor(out=p7[:].rearrange("p (g c m) -> p g c m", g=g_per, c=N),
                                    in0=v_t[:].rearrange("p (g m c) -> p g c m", g=g_per, m=N),
                                    in1=u_t[:].rearrange("p (g m) -> p g m", g=g_per).unsqueeze(2).broadcast_to([P, g_per, N, N]),
                                    op=ALU.mult)
            uv = vtile(N, "uv")
            nc.vector.tensor_reduce(out=uv[:].rearrange("p (g c) -> p g c", g=g_per),
                                    in_=p7[:].rearrange("p (g c m) -> p g c m", g=g_per, c=N),
                                    axis=AX.X, op=ALU.add)
            uq = vtile(N, "uq")
            nc.gpsimd.tensor_tensor(out=uq[:], in0=u_t[:], in1=qs[:], op=ALU.mult)
            pvd = vtile(49, "pvd")
            nc.gpsimd.tensor_tensor(out=pvd[:].rearrange("p (g c m) -> p g c m", g=g_per, c=N),
                                    in0=sm3[:, :, 70:SMW].rearrange("p g (c r) -> p g c r", c=N)[:, :, :, 49:56],
                                    in1=uq[:].rearrange("p (g m) -> p g m", g=g_per).unsqueeze(2).broadcast_to([P, g_per, N, N]),
                                    op=ALU.mult)
            uvd = vtile(N, "uvd")
            nc.vector.tensor_reduce(out=uvd[:].rearrange("p (g c) -> p g c", g=g_per),
                                    in_=pvd[:].rearrange("p (g c m) -> p g c m", g=g_per, c=N),
                                    axis=AX.X, op=ALU.add)
            uv2 = vtile(N, "uv2")
            nc.vector.tensor_tensor(out=uv2[:], in0=uv[:], in1=uvd[:], op=ALU.add)
            # coriolis c = Lw + Ldtu - uV
            c1 = vtile(N, "c1")
            nc.vector.tensor_tensor(out=c1[:], in0=lw[:], in1=ldtu2[:], op=ALU.add)
            cor = vtile(N, "cor")
            nc.vector.tensor_tensor(out=cor[:], in0=c1[:], in1=uv2[:], op=ALU.subtract)
            # Hm[i,k] = sum_j L[i,j] L[k,j]  (per-group)
            p8 = vtile(343, "p8")
            p8v = p8[:].rearrange("p (g x) -> p g x", g=g_per)
            for g in range(g_per):
                Lg = sm3[:, g, 0:49].rearrange("p (i j) -> p i j", i=N)
                nc.gpsimd.tensor_tensor(out=p8v[:, g, :].rearrange("p (i k j) -> p i k j", i=N, k=N),
                                        in0=Lg.unsqueeze(2).broadcast_to([P, N, N, N]),
                                        in1=sm3[:, g, 0:49].rearrange("p (k j) -> p k j", k=N).unsqueeze(1).broadcast_to([P, N, N, N]),
                                        op=ALU.mult)
            hm_t = vtile(49, "hm_t")
            nc.vector.tensor_reduce(out=hm_t[:].rearrange("p (g i k) -> p g i k", g=g_per, i=N),
                                    in_=p8[:].rearrange("p (g i k j) -> p g i k j", g=g_per, i=N, k=N),
                                    axis=AX.X, op=ALU.add)
            # tauH[i] = sum_k Hm[i,k] qdd[k]
            p9 = vtile(49, "p9")
            nc.gpsimd.tensor_tensor(out=p9[:].rearrange("p (g i k) -> p g i k", g=g_per, i=N),
                                    in0=hm_t[:].rearrange("p (g i k) -> p g i k", g=g_per, i=N),
                                    in1=qdd3.unsqueeze(2).broadcast_to([P, g_per, N, N]),
                                    op=ALU.mult)
            tauh = vtile(N, "tauh")
            nc.vector.tensor_reduce(out=tauh[:].rearrange("p (g i) -> p g i", g=g_per),
                                    in_=p9[:].rearrange("p (g i k) -> p g i k", g=g_per, i=N),
                                    axis=AX.X, op=ALU.add)
            # friction (sigmoid/tanh window for th)
            th = vtile(N, "th")
            nc.scalar.activation(th[:], qdsm[:, 0:w7], AF.Tanh, scale=100.0)
            t3 = vtile(N, "t3")
            nc.gpsimd.tensor_tensor(out=t3[:].rearrange("p (g n) -> p g n", g=g_per),
                                    in0=ee[:].rearrange("p (g n) -> p g n", g=g_per),
                                    in1=fr["fs"], op=ALU.mult)
            t4 = vtile(N, "t4")
            nc.gpsimd.tensor_tensor(out=t4[:].rearrange("p (g n) -> p g n", g=g_per),
                                    in0=t3[:].rearrange("p (g n) -> p g n", g=g_per),
                                    in1=fr["fc"], op=ALU.add)
            t5 = vtile(N, "t5")
            nc.gpsimd.tensor_tensor(out=t5[:], in0=t4[:], in1=th[:], op=ALU.mult)
            t6 = vtile(N, "t6")
            nc.gpsimd.tensor_tensor(out=t6[:].rearrange("p (g n) -> p g n", g=g_per),
                                    in0=qdsm[:, 0:w7].rearrange("p (g n) -> p g n", g=g_per),
                                    in1=fr["fd"], op=ALU.mult)
            fric = vtile(N, "fric")
            nc.gpsimd.tensor_tensor(out=fric[:], in0=t5[:], in1=t6[:], op=ALU.add)
            # tau = tauH + cor + g + fric
            s1 = vtile(N, "s1")
            nc.vector.tensor_tensor(out=s1[:], in0=tauh[:], in1=cor[:], op=ALU.add)
            s2 = vtile(N, "s2")
            nc.vector.tensor_tensor(out=s2[:].rearrange("p (g n) -> p g n", g=g_per),
                                    in0=s1[:].rearrange("p (g n) -> p g n", g=g_per),
                                    in1=sm3[:, :, 49:56], op=ALU.add)
            tau = vtile(N, "tau")
            nc.vector.tensor_tensor(out=tau[:], in0=s2[:], in1=fric[:], op=ALU.add)

            # ---------------- outputs ---------------------------------------
            def store(dst, src_ap):
                nc.sync.dma_start(
                    dr[dst].ap()[r0:r0 + ns, :].rearrange("(p g) n -> p (g n)", g=g_per), src_ap)

            store("o_tau", tau[:])
            store("o_c", cor[:])
            store("o_fric", fric[:])
            store("o_hm", hm_t[:])
            store("o_g", sm3[:, :, 49:56])
            store("o_ldraw", sm3[:, :, 56:63])

        if repeat is not None:
            with tc.For_i(0, repeat, 1):
                for t in range(nt):
                    body(t)
        else:
            for t in range(nt):
                body(t)

    nc.compile()
    return nc


def _host_inputs(q, qd, qdd, W0, b0, W1, b1, W2, b2, Wg, bg, Wld, bld, Wlt, blt,
                 fd, fc, fs, fv, bc=BC, ns=NS):
    f = np.float32
    W0, W1, W2 = np.asarray(W0, f), np.asarray(W1, f), np.asarray(W2, f)
    Wg, Wld, Wlt = np.asarray(Wg, f), np.asarray(Wld, f), np.asarray(Wlt, f)
    rows, cols = np.tril_indices(N, -1)
    s_ld = np.zeros((N, 70), f)
    s_sld = np.zeros((N, 70), f)
    s_bias = np.zeros((1, 70), f)
    for d in range(N):
        s_ld[d, 8 * d] = -1.0          # diag = softplus = -(ldspn)
        s_sld[d, 63 + d] = 1.0
        s_bias[0, 8 * d] = 1e-3
        s_bias[0, 49 + d] = np.float32(bg[d])
        s_bias[0, 56 + d] = np.float32(bld[d])
    for ti in range(NTRIL):
        s_bias[0, 7 * rows[ti] + cols[ti]] = np.float32(blt[ti])
    # folded forward head: rows of Lx[0:63] = [-Wlt at tril | -Wg | -Wld]
    wlx = np.zeros((70, HID), f)
    for ti in range(NTRIL):
        wlx[7 * rows[ti] + cols[ti], :] = -Wlt[ti, :]
    wlx[49:56, :] = -Wg
    wlx[56:63, :] = -Wld
    # head-jacobian: [Wlt expanded to the full 49 L-rows | Wld] (true sign)
    whj = np.zeros((DBW, HID), f)
    for ti in range(NTRIL):
        whj[7 * rows[ti] + cols[ti], :] = Wlt[ti, :]
    whj[49:56, :] = Wld
    fricC = np.concatenate([
        np.tile(np.asarray(fd, f)[None, :], (128, 1)),
        np.tile(np.asarray(fc, f)[None, :], (128, 1)),
        np.tile(np.asarray(fs, f)[None, :], (128, 1)),
        np.tile((-1.0 / np.maximum(np.asarray(fv, f), 1e-3))[None, :], (128, 1)),
    ], axis=1).astype(f)

    shared = {
        "w0t": np.ascontiguousarray(W0.T),
        "w0c": np.ascontiguousarray(W0),
        "w1tn": np.ascontiguousarray(-W1.T),
        "w2tn": np.ascontiguousarray(-W2.T),
        "w1tj": np.ascontiguousarray(W1.T),
        "w2tj": np.ascontiguousarray(W2.T),
        "wldtn": np.ascontiguousarray(-Wld.T),
        "wlxt": np.ascontiguousarray(wlx.T),
        "whjt": np.ascontiguousarray(whj.T),
        "b0": np.asarray(b0, f), "b1": np.asarray(b1, f), "b2": np.asarray(b2, f),
        "bld": np.asarray(bld, f),
        "s_ld": s_ld, "s_sld": s_sld, "s_bias": s_bias,
        "ident": np.eye(128, dtype=f),
        "ones_row": np.ones((1, ns), f),
        "fricC": fricC,
    }
    n_cores = (q.shape[0] + bc - 1) // bc
    in_maps = []
    for ci in range(n_cores):
        sl = slice(ci * bc, (ci + 1) * bc)
        m = dict(shared)
        m["q"] = np.ascontiguousarray(np.asarray(q, f)[sl])
        m["qd"] = np.ascontiguousarray(np.asarray(qd, f)[sl])
        m["qdd"] = np.ascontiguousarray(np.asarray(qdd, f)[sl])
        in_maps.append(m)
    return in_maps


_NC_CACHE = {}


def _get_program(bc=BC, ns=NS, repeat=None):
    key = (bc, ns, repeat)
    if key not in _NC_CACHE:
        _NC_CACHE[key] = _build_program(bc, ns, repeat)
    return _NC_CACHE[key]


def kernel(q, qd, qdd, W0, b0, W1, b1, W2, b2, Wg, bg, Wld, bld, Wlt, blt,
           fd, fc, fs, fv):
    from concourse.bass_utils import run_bass_kernel_spmd

    nc = _get_program()
    in_maps = _host_inputs(q, qd, qdd, W0, b0, W1, b1, W2, b2, Wg, bg, Wld, bld,
                           Wlt, blt, fd, fc, fs, fv)
    res = run_bass_kernel_spmd(nc, in_maps, list(range(N_CORES)))
    outs = {nm: np.concatenate([r[nm] for r in res.results], axis=0)
            for nm in ("o_tau", "o_hm", "o_c", "o_g", "o_fric", "o_ldraw")}
    B = q.shape[0]
    return (outs["o_tau"], outs["o_hm"].reshape(B, N, N), outs["o_c"],
            outs["o_g"], outs["o_fric"], outs["o_ldraw"])
